# revision 14
# baseline (speedup 1.0000x reference)
"""Trainium2 Bass kernel for nn_ActivationUnit (DIN-style attention unit).

Per field f (F=8): h = cat([key,q,q-key]) @ W_h[f] + b_h  ->  Dice BN (global
batch stats) -> sigmoid gate -> scores = h' @ W_o[f] + b_o -> masked softmax
over S -> attention-weighted sum of key.

Algebra used here: cat @ W_h = key @ (W0-W2) + q @ (W1+W2); the q-term is
constant over S and injected via a tiled-identity matmul. Data-parallel over
B across 8 cores; BN stats combined with one small AllGather.

Layouts (per core, BL=32 batches):
  keyT  [pair, 128, 6400]  d-on-partitions, cols s-major (col = s*32+b),
                           two fields stacked (K-packing, block-diag A2)
  nat   [100, 32768]       natural key rows; chunk i=(b*16+2f+half) at cols
                           64i..64i+64, partitions = s within half
  h     [72, 6400]         h^T for a field pair (j of field0 | j of field1)
  pairs [128, 200]         softmax layout, partition p = f*16 + (b%16)
"""
import os
import contextlib
import numpy as np
import ml_dtypes

import concourse.bass as bass
import concourse.bacc as bacc
import concourse.tile as tile
from concourse import mybir
from concourse.bass_utils import run_bass_kernel_spmd

BF16 = ml_dtypes.bfloat16

N_CORES = 8
B, F, S, D, H = 256, 8, 200, 64, 36
BL = B // N_CORES            # 32
COLS = BL * S                # 6400
NCH = 320                    # matmul chunk (10 s-values * 32 b)
PCH = 640                    # psum h-tile cols (2 chunks, 2 banks)
NPT = COLS // PCH            # 10
NPAIR = F // 2               # 4
H2 = 2 * H                   # 72
EPS = 1e-3

_cached = {}


def _build_program(use_alpha: bool):
    nc = bacc.Bacc("TRN2", target_bir_lowering=False, debug=False,
                   num_devices=N_CORES)
    f32 = mybir.dt.float32
    bf16 = mybir.dt.bfloat16
    AF = mybir.ActivationFunctionType
    OP = mybir.AluOpType

    keyT = nc.dram_tensor("keyT", [NPAIR, 128, COLS], bf16, kind="ExternalInput").ap()
    nat = nc.dram_tensor("nat", [100, BL * F * 2 * D], bf16, kind="ExternalInput").ap()
    Esm = nc.dram_tensor("Esm", [BL, 544], bf16, kind="ExternalInput").ap()
    qTaug = nc.dram_tensor("qTaug", [D + 1, F * BL], f32, kind="ExternalInput").ap()
    Caug = nc.dram_tensor("Caug", [D + 1, F * H], f32, kind="ExternalInput").ap()
    A2d = nc.dram_tensor("A2", [128, NPAIR * H2], bf16, kind="ExternalInput").ap()
    Wo2d = nc.dram_tensor("Wo2", [H2, NPAIR * 2], bf16, kind="ExternalInput").ap()
    maskd = nc.dram_tensor("maskb", [128, 2 * S], bf16, kind="ExternalInput").ap()
    eyed = nc.dram_tensor("eye128", [128, 128], bf16, kind="ExternalInput").ap()
    if use_alpha:
        ald = nc.dram_tensor("al72", [H2, NPAIR * 2], f32, kind="ExternalInput").ap()
    y = nc.dram_tensor("y", [BL, F, D], f32, kind="ExternalOutput").ap()
    debug = os.environ.get("NN_DEBUG") == "1"
    if debug:
        dbg_h = nc.dram_tensor("dbg_h", [NPAIR, H2, COLS], bf16, kind="ExternalOutput").ap()
        dbg_s = nc.dram_tensor("dbg_s", [F, BL, S], bf16, kind="ExternalOutput").ap()

    with tile.TileContext(nc) as tc:
        ctx = contextlib.ExitStack()
        with ctx:
            big = ctx.enter_context(tc.tile_pool(name="big", bufs=4))
            store = ctx.enter_context(tc.tile_pool(name="store", bufs=1))
            work = ctx.enter_context(tc.tile_pool(name="work", bufs=2))
            small = ctx.enter_context(tc.tile_pool(name="small", bufs=1))
            psum = ctx.enter_context(tc.tile_pool(name="psum", bufs=2, space="PSUM"))
            psc = ctx.enter_context(tc.tile_pool(name="psc", bufs=3, space="PSUM"))
            dram = ctx.enter_context(tc.tile_pool(name="dram", bufs=1, space="DRAM"))

            # ------- small loads -------
            sb_E = small.tile([BL, 544], bf16)
            nc.sync.dma_start(out=sb_E, in_=Esm)
            sb_A2 = small.tile([128, NPAIR, H2], bf16)
            nc.sync.dma_start(out=sb_A2.rearrange("k p m -> k (p m)"), in_=A2d)
            sb_Wo2 = small.tile([H2, NPAIR * 2], bf16)
            nc.sync.dma_start(out=sb_Wo2, in_=Wo2d)
            sb_qT = small.tile([D + 1, F * BL], f32)
            nc.sync.dma_start(out=sb_qT, in_=qTaug)
            sb_C = small.tile([D + 1, F * H], f32)
            nc.sync.dma_start(out=sb_C, in_=Caug)
            sb_mask = small.tile([128, 2, S], bf16)
            nc.sync.dma_start(out=sb_mask.rearrange("p t s -> p (t s)"), in_=maskd)
            sb_eye = small.tile([128, 128], bf16)
            nc.sync.dma_start(out=sb_eye, in_=eyed)
            if use_alpha:
                sb_al = small.tile([H2, NPAIR * 2], f32)
                nc.sync.dma_start(out=sb_al, in_=ald)

            sb_keyT = []
            for pr in range(NPAIR):
                t = big.tile([128, COLS], bf16, tag="big")
                nc.sync.dma_start(out=t, in_=keyT[pr])
                sb_keyT.append(t)

            # ------- qc = q @ (W1+W2) + b_h, computed transposed [b, j] -------
            sb_qcT = small.tile([BL, NPAIR * H2], bf16)
            for f in range(F):
                ps_qc = psc.tile([BL, H], f32, tag="aux")
                nc.tensor.matmul(ps_qc, sb_qT[:, f * BL:(f + 1) * BL],
                                 sb_C[:, f * H:(f + 1) * H], start=True, stop=True)
                nc.vector.tensor_copy(out=sb_qcT[:, f * H:(f + 1) * H], in_=ps_qc)

            # ------- phase 1: h^T tiles + store bf16 + bn_stats -------
            h_st = [store.tile([H2, COLS], bf16, name=f"hst{i}") for i in range(NPAIR)]
            stats6 = small.tile([H2, NPAIR, COLS // NCH, 6], f32)
            # bank-aligned psum chunks: 12 x 512 + 1 x 256
    #        (a matmul output must stay within one 2KB PSUM bank)
            chunks = [512] * 12 + [256]
            cp = 0
            for pr in range(NPAIR):
                c0 = 0
                for w in chunks:
                    ph = psum.tile([H2, 512], f32, tag="hpsum")
                    nc.tensor.matmul(ph[:, 0:w], sb_A2[:, pr, :],
                                     sb_keyT[pr][:, c0:c0 + w],
                                     start=True, stop=False)
                    eo = c0 % BL
                    nc.tensor.matmul(ph[:, 0:w],
                                     sb_qcT[:, pr * H2:(pr + 1) * H2],
                                     sb_E[:, eo:eo + w],
                                     start=False, stop=True)
                    dst = h_st[pr][:, c0:c0 + w]
                    if cp % 2 == 0:
                        nc.scalar.copy(out=dst, in_=ph[:, 0:w])
                    else:
                        nc.vector.tensor_copy(out=dst, in_=ph[:, 0:w])
                    cp += 1
                    c0 += w
                h3 = h_st[pr].rearrange("p (g n) -> p g n", n=NCH)
                for g_i in range(COLS // NCH):
                    nc.vector.bn_stats(out=stats6[:, pr, g_i], in_=h3[:, g_i])

            # ------- local aggregate + AllGather + global stats -------
            ag_in_sb = small.tile([H2, NPAIR * 3], f32)
            for pr in range(NPAIR):
                mv = small.tile([H2, 2], f32, tag=f"mv{pr}")
                nc.vector.bn_aggr(out=mv, in_=stats6[:, pr])
                nc.vector.memset(ag_in_sb[:, pr * 3:pr * 3 + 1], float(COLS))
                nc.gpsimd.tensor_copy(out=ag_in_sb[:, pr * 3 + 1:pr * 3 + 2],
                                      in_=mv[:, 0:1])
                nc.vector.tensor_scalar_mul(ag_in_sb[:, pr * 3 + 2:pr * 3 + 3],
                                            mv[:, 1:2], float(COLS))
            ag_in = dram.tile([H2, NPAIR * 3], f32)
            ag_out = dram.tile([N_CORES * H2, NPAIR * 3], f32, addr_space="Shared")
            nc.gpsimd.dma_start(out=ag_in, in_=ag_in_sb)
            nc.gpsimd.collective_compute(
                "AllGather", OP.bypass,
                replica_groups=[list(range(N_CORES))],
                ins=[ag_in[:]], outs=[ag_out[:]])
            gath = small.tile([H2, N_CORES, NPAIR * 3], f32)
            nc.sync.dma_start(
                out=gath, in_=ag_out.rearrange("(c p) m -> p c m", p=H2))

            scl, bia = [], []
            sb_eps = small.tile([H2, 1], f32)
            nc.vector.memset(sb_eps, EPS)
            for pr in range(NPAIR):
                mvg = small.tile([H2, 2], f32, tag=f"mvg{pr}")
                nc.vector.bn_aggr(out=mvg, in_=gath[:, :, pr * 3:(pr + 1) * 3])
                s_t = small.tile([H2, 1], f32, tag=f"s{pr}")
                b_t = small.tile([H2, 1], f32, tag=f"b{pr}")
                nc.scalar.activation(out=s_t, in_=mvg[:, 1:2], func=AF.Sqrt,
                                     bias=sb_eps, scale=1.0)
                nc.vector.reciprocal(out=s_t, in_=s_t)
                nc.vector.tensor_tensor(out=b_t, in0=mvg[:, 0:1], in1=s_t,
                                        op=OP.mult)
                nc.vector.tensor_scalar_mul(b_t, b_t, -1.0)
                scl.append(s_t)
                bia.append(b_t)

            # ------- phase 2: sigmoid + dice (in place) -------
            for pr in range(NPAIR):
                p_t = work.tile([H2, COLS], bf16, tag="p")
                nc.scalar.activation(out=p_t, in_=h_st[pr], func=AF.Sigmoid,
                                     bias=bia[pr], scale=scl[pr])
                if use_alpha:
                    nc.vector.tensor_scalar(
                        out=p_t, in0=p_t,
                        scalar1=sb_al[:, pr * 2:pr * 2 + 1],
                        scalar2=sb_al[:, pr * 2 + 1:pr * 2 + 2],
                        op0=OP.mult, op1=OP.add)
                nc.vector.tensor_tensor(out=h_st[pr], in0=p_t, in1=h_st[pr],
                                        op=OP.mult)

            # ------- scores: diced @ Wo2 -> psum -> sbuf -> DRAM (f,b,s) -------
            scp = dram.tile([F, BL, S], bf16)
            for ch in range(COLS // NCH):
                ps_s = psc.tile([98, NCH], f32, tag="aux")
                for pr in range(NPAIR):
                    nc.tensor.matmul(ps_s[32 * pr:32 * pr + 2, :],
                                     sb_Wo2[:, pr * 2:(pr + 1) * 2],
                                     h_st[pr][:, ch * NCH:(ch + 1) * NCH],
                                     start=True, stop=True,
                                     tile_position=(0, 32 * pr))
                roll = work.tile([98, NCH], bf16, tag="roll")
                # evac psum -> roll with cols rearranged to b-major (b*10+s)
                nsub = NCH // BL
                nc.scalar.copy(
                    out=roll.rearrange("p (b s) -> p s b", s=nsub),
                    in_=ps_s.rearrange("p (s b) -> p s b", b=BL))
                for pr in range(NPAIR):
                    nc.sync.dma_start(
                        out=scp[2 * pr:2 * pr + 2, :, ch * nsub:(ch + 1) * nsub],
                        in_=roll[32 * pr:32 * pr + 2, :].rearrange(
                            "p (b s) -> p b s", s=nsub))

            if debug:
                nc.sync.dma_start(out=dbg_s, in_=scp[:])
                for pr in range(NPAIR):
                    nc.sync.dma_start(out=dbg_h[pr], in_=h_st[pr])

            # ------- softmax in pairs layout (p = f*16 + b%16) -------
            attn_t = []
            for t in range(2):
                pairs = work.tile([128, S], bf16, tag=f"pairs{t}")
                for f in range(F):
                    nc.sync.dma_start(out=pairs[f * 16:(f + 1) * 16, :],
                                      in_=scp[f, 16 * t:16 * (t + 1), :])
                mk = work.tile([128, S], bf16, tag=f"mk{t}")
                nc.vector.tensor_tensor(out=mk, in0=pairs, in1=sb_mask[:, t, :],
                                        op=OP.add)
                nmax = work.tile([128, 1], f32, tag=f"nm{t}")
                nc.vector.reduce_max(out=nmax, in_=mk, axis=mybir.AxisListType.X,
                                     negate=True)
                e = work.tile([128, S], bf16, tag=f"e{t}")
                z = work.tile([128, 1], f32, tag=f"z{t}")
                nc.scalar.activation(out=e, in_=mk, func=AF.Exp,
                                     bias=nmax, scale=1.0, accum_out=z)
                rz = work.tile([128, 1], f32, tag=f"rz{t}")
                nc.vector.reciprocal(out=rz, in_=z)
                attn = work.tile([128, S], bf16, tag=f"at{t}")
                nc.vector.tensor_scalar_mul(attn, e, rz)
                attn_t.append(attn)

            # ------- attn transpose + block-diag lhs fill -------
            lhsC = store.tile([100, BL * 128], bf16)
            nc.vector.memset(lhsC, 0.0)
            attnT = small.tile([100, 512], bf16)   # cols: half*256 + t*128 + f*16+bb
            for half in range(2):
                for t in range(2):
                    ps_t = psc.tile([100, 128], bf16, tag="aux")
                    nc.tensor.transpose(
                        ps_t, attn_t[t][:, half * 100:(half + 1) * 100], sb_eye)
                    nc.vector.tensor_copy(
                        out=attnT[:, half * 256 + t * 128:half * 256 + (t + 1) * 128],
                        in_=ps_t)
            lhsC3 = lhsC.rearrange("p (b c) -> p b c", c=128)
            for half in range(2):
                for t in range(2):
                    for f in range(F):
                        c0 = half * 256 + t * 128 + f * 16
                        col = f * 17 + half * 8
                        nc.sync.dma_start(
                            out=lhsC3[:, 16 * t:16 * (t + 1), col:col + 1],
                            in_=attnT[:, c0:c0 + 16].rearrange("p (b o) -> p b o", o=1))

            # ------- phase C: weighted sums via block-diag matmuls -------
            sb_nat = [big.tile([100, 8192], bf16, tag="big", name=f"nat{_i}") for _i in range(4)]
            for i in range(4):
                nc.sync.dma_start(out=sb_nat[i], in_=nat[:, i * 8192:(i + 1) * 8192])
            out_sb = store.tile([128, F * D], f32)
            for g in range(F):
                ps_o = psc.tile([128, D], f32, tag="aux")
                for k in range(4):
                    b_idx = g * 4 + k
                    for cchunk in range(16):
                        gi = b_idx * 16 + cchunk
                        nti, off = gi // 128, (gi % 128) * 64
                        nc.tensor.matmul(
                            ps_o[32 * k:32 * k + F, :],
                            lhsC[:, gi * 8:(gi + 1) * 8],
                            sb_nat[nti][:, off:off + 64],
                            start=(cchunk == 0), stop=(cchunk == 15),
                            tile_position=(0, 32 * k))
                nc.scalar.copy(out=out_sb[0:104, g * D:(g + 1) * D],
                               in_=ps_o[0:104, :])

            # ------- output -------
            y2 = y.rearrange("(g k) f d -> k f g d", k=4)
            for k in range(4):
                nc.sync.dma_start(
                    out=y2[k],
                    in_=out_sb[32 * k:32 * k + F, :].rearrange(
                        "p (g d) -> p g d", d=D))

    nc.compile()
    return nc


def _host_prep(query, key, W_h, b_h, alpha, W_o, b_o, sequence_num):
    A = (W_h[:, 0:D, :] - W_h[:, 2 * D:3 * D, :]).astype(np.float32)
    C = (W_h[:, D:2 * D, :] + W_h[:, 2 * D:3 * D, :]).astype(np.float32)
    A2 = np.zeros((128, NPAIR * H2), np.float32)
    Wo2 = np.zeros((H2, NPAIR * 2), np.float32)
    al72 = np.zeros((H2, NPAIR * 2), np.float32)
    for pr in range(NPAIR):
        A2[0:64, pr * H2:pr * H2 + H] = A[2 * pr]
        A2[64:128, pr * H2 + H:pr * H2 + H2] = A[2 * pr + 1]
        Wo2[0:H, pr * 2] = W_o[2 * pr, :, 0]
        Wo2[H:H2, pr * 2 + 1] = W_o[2 * pr + 1, :, 0]
        al72[0:H, pr * 2] = 1.0 - alpha[2 * pr]
        al72[0:H, pr * 2 + 1] = alpha[2 * pr]
        al72[H:H2, pr * 2] = 1.0 - alpha[2 * pr + 1]
        al72[H:H2, pr * 2 + 1] = alpha[2 * pr + 1]
    Caug0 = np.concatenate([C, b_h[:, None, :]], axis=1)          # [F,65,36]
    Caug = Caug0.transpose(1, 0, 2).reshape(D + 1, F * H)
    Esm = np.tile(np.eye(BL, dtype=np.float32), (1, 17))[:, :544]
    eye128 = np.eye(128, dtype=np.float32)

    in_maps = []
    for c in range(N_CORES):
        sl = slice(c * BL, (c + 1) * BL)
        k_sh = np.ascontiguousarray(key[sl])           # [32,8,200,64]
        q_sh = np.ascontiguousarray(query[sl])         # [32,8,64]
        sq = sequence_num[sl]
        t = k_sh.transpose(1, 3, 2, 0)                 # [f,d,s,b]
        keyT_f = t.reshape(F, D, COLS)
        keyT_p = np.concatenate([keyT_f[0::2], keyT_f[1::2]], axis=1)
        natm = (k_sh.reshape(BL, F, 2, 100, D)
                .transpose(3, 0, 1, 2, 4).reshape(100, -1))
        qT = q_sh.transpose(1, 2, 0).astype(np.float32)
        qTaug = np.concatenate([qT, np.ones((F, 1, BL), np.float32)], axis=1)
        qTaug = qTaug.transpose(1, 0, 2).reshape(D + 1, F * BL)
        mb = np.zeros((128, 2, S), np.float32)
        s_i = np.arange(S)
        for tt in range(2):
            for f in range(F):
                for bb in range(16):
                    valid = s_i < sq[tt * 16 + bb, f]
                    mb[f * 16 + bb, tt] = np.where(valid, b_o[f, 0], -1e30)
        in_maps.append({
            "keyT": keyT_p.astype(BF16),
            "nat": natm.astype(BF16),
            "Esm": Esm.astype(BF16),
            "qTaug": qTaug,
            "Caug": Caug,
            "A2": A2.astype(BF16),
            "Wo2": Wo2.astype(BF16),
            "maskb": mb.reshape(128, 2 * S).astype(BF16),
            "eye128": eye128.astype(BF16),
            "al72": al72,
        })
    return in_maps


def kernel(query, key, W_h, b_h, alpha, W_o, b_o, sequence_num, training=1):
    query = np.asarray(query, np.float32)
    key = np.asarray(key, np.float32)
    W_h = np.asarray(W_h, np.float32)
    b_h = np.asarray(b_h, np.float32)
    alpha = np.asarray(alpha, np.float32)
    W_o = np.asarray(W_o, np.float32)
    b_o = np.asarray(b_o, np.float32).reshape(F, 1)
    sequence_num = np.asarray(sequence_num)

    use_alpha = bool(np.any(alpha != 0.0))
    want_trace = os.environ.get("NN_TRACE") == "1"
    if not want_trace:
        os.environ.setdefault("BASS_NEVER_TRACE", "1")

    ck = ("prog", use_alpha)
    if ck not in _cached:
        _cached[ck] = _build_program(use_alpha)
    nc = _cached[ck]

    in_maps = _host_prep(query, key, W_h, b_h, alpha, W_o, b_o, sequence_num)
    if not use_alpha:
        for m in in_maps:
            m.pop("al72")

    res = run_bass_kernel_spmd(nc, in_maps, core_ids=list(range(N_CORES)),
                               trace=want_trace)
    kernel._last_exec_ns = res.exec_time_ns
    kernel._last_results = res
    out = np.concatenate([res.results[c]["y"] for c in range(N_CORES)], axis=0)
    return out.astype(np.float32)


kernel._last_exec_ns = None
